# revision 26
# baseline (speedup 1.0000x reference)
"""Trainium2 Bass kernel for nn_DMlp_46823733461564 (dense_mlp).

Computes: token-grid 3x3 masked-neighborhood gather (pixel-shuffle +
reflection-pad + masked unfold, algebraically reduced to a channel-
permuted shifted gather) followed by fc1(1600->1024) + exact GELU +
fc2(1024->576).

Sharding: data-parallel over (batch, image-half) -> 8 cores, 8192 tokens
each; fc weights replicated. The gather runs on-device as strided DMAs
from a host-prepared reflection-extended channel-major image.

fc1 K-dim mixed precision: 192 of the 1600 K-rows (taps 0,1,24) run as
one fp8-e4m3 DoubleRow matmul pass (2 K-streams/cycle); the remaining
1408 K-rows (taps 2..23, 11 full 128-chunks) run in bf16. fc2 in bf16.
All w1 scaled x64 host-side (fp8 subnormal avoidance), undone by the
GELU activation's scale. Measured end-to-end rel err ~1.5e-2 (gate 2e-2).
"""
import os
import sys

import numpy as np

_TRN_REPO = "/opt/trn_rl_repo"
if _TRN_REPO not in sys.path:
    sys.path.insert(0, _TRN_REPO)

B, HIMG, WIMG = 4, 128, 128
C = 64
L = 576           # C * 9
NTOK = HIMG * WIMG
HID = 1024
OUTF = 576
INF = 1600        # C * 25
N_CORES = 8
ROWS_PER_CORE = HIMG // 2          # 64 token rows
TOK_PER_CORE = ROWS_PER_CORE * WIMG  # 8192
TILE_ROWS = 4                      # image rows per token tile
TT = TILE_ROWS * WIMG              # 512 tokens per tile
N_TILES = ROWS_PER_CORE // TILE_ROWS  # 16
KC_BF = 10                         # bf16 K-chunks (taps 4..23)
W1SCALE = 64.0

_MASK = np.array([
    [1, 0, 0, 1, 0, 0, 1],
    [0, 1, 0, 1, 0, 1, 0],
    [0, 0, 1, 1, 1, 0, 0],
    [1, 1, 1, 1, 1, 1, 1],
    [0, 0, 1, 1, 1, 0, 0],
    [0, 1, 0, 1, 0, 1, 0],
    [1, 0, 0, 1, 0, 0, 1]], dtype=bool)
MASK_POS = [(i, j) for i in range(7) for j in range(7) if _MASK[i, j]]


def _dmap(d):
    if d <= 1:
        return -1, d + 1
    if d <= 4:
        return 0, d - 2
    return 1, d - 5


KPOS = []
for (_di, _dj) in MASK_POS:
    _dh, _r1 = _dmap(_di)
    _dw, _r2 = _dmap(_dj)
    KPOS.append((_dh, _dw, _r1 * 3 + _r2))


def _swap_map(a, b, which):
    ch = np.arange(L)
    c, rem = ch // 9, ch % 9
    r1, r2 = rem // 3, rem % 3
    r = r1 if which == 0 else r2
    rs = np.where(r == a, b, np.where(r == b, a, r))
    if which == 0:
        return c * 9 + rs * 3 + r2
    return c * 9 + r1 * 3 + rs


def _build_xe(x):
    """x: (B, NTOK, L) -> xe: (B, L, HIMG+2, WIMG+2) reflection-extended,
    channel-permuted borders."""
    xt = np.ascontiguousarray(x.transpose(0, 2, 1)).reshape(B, L, HIMG, WIMG)
    xe = np.empty((B, L, HIMG + 2, WIMG + 2), dtype=np.float32)
    xe[:, :, 1:-1, 1:-1] = xt
    xe[:, :, 0, 1:-1] = xt[:, _swap_map(1, 2, 0), 0, :]
    xe[:, :, -1, 1:-1] = xt[:, _swap_map(0, 1, 0), -1, :]
    xe[:, :, :, 0] = np.take(xe[:, :, :, 1], _swap_map(1, 2, 1), axis=1)
    xe[:, :, :, -1] = np.take(xe[:, :, :, -2], _swap_map(0, 1, 1), axis=1)
    return xe


_NC_CACHE = {}

USE_FP8 = os.environ.get("KERNEL_FP8", "1") == "1"
MODE = "bf16+fp8" if USE_FP8 else "bf16"

_WS_COUNTER = [0]


def _split_waits(nc, limit=1):
    """walrus in this toolchain accepts only ONE sync wait per instruction;
    move excess waits onto same-engine NoOps inserted just before (engine
    program order makes this equivalent)."""
    import concourse.mybir as mybir

    def noop(engine, waits):
        _WS_COUNTER[0] += 1
        return mybir.InstNoOp(
            name=f"WS-{_WS_COUNTER[0]}",
            sync_info=mybir.SyncInfo(on_wait=list(waits), on_update=[]),
            bass_nofuse=True,
            engine=engine,
        )

    for fn in nc.m.functions:
        for blk in fn.blocks:
            new_insts = []
            for inst in blk.instructions:
                si = getattr(inst, "sync_info", None)
                waits = list(si.on_wait) if si and si.on_wait else []
                if len(waits) > limit:
                    excess = waits[: len(waits) - limit]
                    si.on_wait = waits[len(waits) - limit:]
                    while excess:
                        new_insts.append(noop(inst.engine, excess[:limit]))
                        excess = excess[limit:]
                new_insts.append(inst)
            blk.instructions = new_insts


def _build_bass():
    if "nc" in _NC_CACHE:
        return _NC_CACHE["nc"]
    import concourse.bass as bass
    import concourse.mybir as mybir
    from concourse.tile import TileContext

    f32 = mybir.dt.float32
    bf16 = mybir.dt.bfloat16
    fp8 = mybir.dt.float8e4
    AF = mybir.ActivationFunctionType
    Alu = mybir.AluOpType
    DR = mybir.MatmulPerfMode.DoubleRow

    nc = bass.Bass("TRN2", target_bir_lowering=False, debug=False)
    xp = nc.dram_tensor("xp", (KC_BF, 128, ROWS_PER_CORE * WIMG), bf16,
                        kind="ExternalInput")
    w1p = nc.dram_tensor("w1p", (KC_BF, 128, HID), bf16, kind="ExternalInput")
    w2t = nc.dram_tensor("w2t", (HID // 128, 128, OUTF), bf16,
                         kind="ExternalInput")
    b1rs = nc.dram_tensor("b1rs", (128, HID // 128), f32, kind="ExternalInput")
    b2bc = nc.dram_tensor("b2bc", (128, OUTF), f32, kind="ExternalInput")
    if USE_FP8:
        xq = nc.dram_tensor("xq", (2, 128, ROWS_PER_CORE * WIMG), fp8,
                            kind="ExternalInput")
        xqb = nc.dram_tensor("xqb", (2, 64, ROWS_PER_CORE * WIMG), fp8,
                             kind="ExternalInput")
        w18 = nc.dram_tensor("w18", (128, 2, HID), fp8, kind="ExternalInput")
        w18b = nc.dram_tensor("w18b", (64, 2, HID), fp8, kind="ExternalInput")
    out = nc.dram_tensor("out", (TOK_PER_CORE, OUTF), f32,
                         kind="ExternalOutput")

    with TileContext(nc) as tc:
        with (
            tc.tile_pool(name="wpool", bufs=1) as wpool,
            tc.tile_pool(name="fpool", bufs=3) as fpool,
            tc.tile_pool(name="hpool", bufs=2) as hpool,
            tc.tile_pool(name="opool", bufs=3) as opool,
            tc.tile_pool(name="ps1", bufs=2, space="PSUM") as ps1,
            tc.tile_pool(name="ps2", bufs=2, space="PSUM") as ps2,
        ):
            # --- PE warmup: dependency-free matmuls fill the initial DMA
            # wait and push the HAM clock gate to 8/8 before real work ---
            n_warm = int(os.environ.get("KERNEL_WARMUP", "320"))
            warm_n = int(os.environ.get("KERNEL_WARMUP_N", "128"))
            if n_warm:
                warm = wpool.tile([128, 512], bf16, tag="warm")
                nc.vector.memset(warm[:, :], 0.0)
                wps = ps2.tile([128, 512], f32, tag="poa")
                for _ in range(n_warm):
                    nc.tensor.matmul(wps[:, 0:warm_n], warm[:, 0:128],
                                     warm[:, 0:warm_n], start=True, stop=True)
            # --- replicated weights, loaded once. Merged into few DMA
            # instructions (each dma_start costs ~0.6us of queue time) and
            # spread over the idle gpsimd/vector queues so the streams run
            # in parallel and never crowd the gather queues ---
            # --- tile-0 feat gathers first: per-chunk DMAs (chunk-granular
            # semaphores let fc1 m=0 start as chunks land), spread over the
            # sync+scalar queues (each dynamic queue streams ~100GB/s) ---
            QS = [nc.sync, nc.scalar, nc.gpsimd]
            t0_fts = []
            for j in range(KC_BF):
                ft = fpool.tile([128, TT], bf16, tag=f"f{j}")
                t0_fts.append(ft)
                QS[j % 2].dma_start(
                    out=ft[:, :].rearrange("p (r w) -> p r w", r=TILE_ROWS),
                    in_=xp[j, :, 0:TT].rearrange(
                        "p (r w) -> p r w", r=TILE_ROWS))
            t0_ft8 = t0_ft8b = None
            if USE_FP8:
                w18t = wpool.tile([128, 2, HID], fp8, tag="w18")
                nc.gpsimd.dma_start(out=w18t[:, :, :], in_=w18[:, :, :])
                w18bt = wpool.tile([64, 2, HID], fp8, tag="w18b")
                nc.gpsimd.dma_start(out=w18bt[:, :, :], in_=w18b[:, :, :])
                t0_ft8 = fpool.tile([128, 2, TT], fp8, tag="f8")
                nc.scalar.dma_start(
                    out=t0_ft8[:, :, :],
                    in_=xq[:, :, 0:TT].rearrange("s p t -> p s t"))
                t0_ft8b = fpool.tile([64, 2, TT], fp8, tag="f8b")
                nc.scalar.dma_start(
                    out=t0_ft8b[:, :, :],
                    in_=xqb[:, :, 0:TT].rearrange("s p t -> p s t"))
            # w1 in half-HID chunks over all three queues, m=0..3 halves
            # first so early fc1 m-groups never wait on late weight bytes
            w1all = wpool.tile([128, KC_BF, HID], bf16, tag="w1")
            HH = HID // 2
            for half in range(2):
                hs = slice(half * HH, half * HH + HH)
                for j in range(KC_BF):
                    QS[(j + half) % 3].dma_start(
                        out=w1all[:, j, hs], in_=w1p[j, :, hs])
            w1sb = [w1all[:, j, :] for j in range(KC_BF)]
            b1t = wpool.tile([128, HID // 128], f32, tag="b1")
            nc.gpsimd.dma_start(out=b1t[:, :], in_=b1rs[:, :])
            # fc2 weights + b2 aren't needed until ~25us in
            w2all = wpool.tile([128, HID // 128, OUTF], bf16, tag="w2")
            w2sb = [w2all[:, j, :] for j in range(HID // 128)]
            b2t = None

            def emit_fc2(hts, r0, tt):
                # --- fc2: out[tok, :] = h.T @ w2t + b2 ---
                NH = OUTF // 2
                ns = tt // 128
                ot = opool.tile([128, ns, OUTF], f32, tag="o")
                for s in range(ns):
                    poa = ps2.tile([128, NH], f32, tag="poa")
                    pob = ps2.tile([128, NH], f32, tag="pob")
                    for j in range(HID // 128):
                        nc.tensor.matmul(
                            poa[:, :],
                            hts[j][:, s * 128: (s + 1) * 128],
                            w2sb[j][:, 0:NH],
                            start=(j == 0), stop=(j == HID // 128 - 1),
                        )
                        nc.tensor.matmul(
                            pob[:, :],
                            hts[j][:, s * 128: (s + 1) * 128],
                            w2sb[j][:, NH:OUTF],
                            start=(j == 0), stop=(j == HID // 128 - 1),
                        )
                    nc.vector.tensor_tensor(
                        out=ot[:, s, 0:NH], in0=poa[:, :], in1=b2t[:, 0:NH],
                        op=Alu.add)
                    nc.vector.tensor_tensor(
                        out=ot[:, s, NH:OUTF], in0=pob[:, :],
                        in1=b2t[:, NH:OUTF], op=Alu.add)
                tok0 = r0 * WIMG
                nc.gpsimd.dma_start(
                    out=out[tok0: tok0 + tt, :].rearrange(
                        "(s p) f -> p s f", s=ns),
                    in_=ot[:, :, :])

            # tile list: 4-row tiles, with the last tile split in half so
            # the serial fc2 tail after the final fc1 is shorter
            tiles = [(i * TILE_ROWS, TILE_ROWS) for i in range(N_TILES - 1)]
            tiles += [((N_TILES - 1) * TILE_ROWS, TILE_ROWS // 2),
                      ((N_TILES - 1) * TILE_ROWS + TILE_ROWS // 2,
                       TILE_ROWS // 2)]

            prev = None  # (hts, r0, tt) of the previous tile: fc2 runs one
            # tile behind fc1 so the PE never waits on the GELU latency
            for t_i, (r0, nrows) in enumerate(tiles):
                tt = nrows * WIMG
                tok = slice(r0 * WIMG, r0 * WIMG + tt)
                if t_i == 0:
                    fts, ft8, ft8b = t0_fts, t0_ft8, t0_ft8b
                    b2t = wpool.tile([128, OUTF], f32, tag="b2")
                    nc.gpsimd.dma_start(out=b2t[:, :], in_=b2bc[:, :])
                    nc.gpsimd.dma_start(
                        out=w2all[:, :, :],
                        in_=w2t[:, :, :].rearrange("j p f -> p j f"))
                else:
                    # steady state: one merged DMA for the 10 bf16 chunks,
                    # one per fp8 slot pair
                    ftall = fpool.tile([128, KC_BF, tt], bf16, tag="f")
                    nc.sync.dma_start(
                        out=ftall[:, :, :],
                        in_=xp[:, :, tok].rearrange("j p t -> p j t"))
                    fts = [ftall[:, j, :] for j in range(KC_BF)]
                    if USE_FP8:
                        ft8 = fpool.tile([128, 2, tt], fp8, tag="f8")
                        nc.scalar.dma_start(
                            out=ft8[:, :, :],
                            in_=xq[:, :, tok].rearrange("s p t -> p s t"))
                        ft8b = fpool.tile([64, 2, tt], fp8, tag="f8b")
                        nc.scalar.dma_start(
                            out=ft8b[:, :, :],
                            in_=xqb[:, :, tok].rearrange("s p t -> p s t"))
                # --- fc1 + GELU: h[m] = gelu((w1.T @ featT)/64 + b1) ---
                hts = []
                for m in range(HID // 128):
                    ps = ps1.tile([128, tt], f32)
                    if USE_FP8:
                        # the two DoubleRow passes are spread apart so each
                        # one's (long, FWL-less) LDWEIGHTS hides under a
                        # full-length bf16 matmul, not under the other DR
                        nc.tensor.matmul(
                            ps[:, :], w18t[:, :, m * 128:(m + 1) * 128],
                            ft8[:, :, :], start=True, stop=False,
                            perf_mode=DR)
                        for j in range(KC_BF):
                            if j == KC_BF // 2:
                                nc.tensor.matmul(
                                    ps[:, :],
                                    w18bt[:, :, m * 128:(m + 1) * 128],
                                    ft8b[:, :, :], start=False, stop=False,
                                    perf_mode=DR)
                            nc.tensor.matmul(
                                ps[:, :], w1sb[j][:, m * 128:(m + 1) * 128],
                                fts[j][:, :],
                                start=False, stop=(j == KC_BF - 1))
                    else:
                        for j in range(KC_BF):
                            nc.tensor.matmul(
                                ps[:, :], w1sb[j][:, m * 128:(m + 1) * 128],
                                fts[j][:, :],
                                start=(j == 0), stop=(j == KC_BF - 1))
                    ht = hpool.tile([128, tt], bf16, tag=f"h{m}")
                    nc.scalar.activation(ht[:, :], ps[:, :], AF.Gelu,
                                         bias=b1t[:, m: m + 1],
                                         scale=1.0 / W1SCALE)
                    hts.append(ht)
                if prev is not None:
                    emit_fc2(*prev)
                prev = (hts, r0, tt)
            emit_fc2(*prev)

    _split_waits(nc)
    _NC_CACHE["nc"] = nc
    return nc


def _host_prep(x, w1, b1, w2, b2):
    import ml_dtypes
    bf16 = ml_dtypes.bfloat16
    fp8 = ml_dtypes.float8_e4m3

    x = np.ascontiguousarray(np.asarray(x, dtype=np.float32))
    w1 = np.asarray(w1, dtype=np.float32)
    b1 = np.asarray(b1, dtype=np.float32)
    w2 = np.asarray(w2, dtype=np.float32)
    b2 = np.asarray(b2, dtype=np.float32)

    xe = _build_xe(x)
    # w1 rows in tap-major order: row k*64+c <- w1.T row c*25+k; scaled x64
    w1t = np.ascontiguousarray(w1.T) * W1SCALE  # (1600, 1024)
    w1tap = np.ascontiguousarray(
        w1t.reshape(C, 25, HID).transpose(1, 0, 2))  # (25, 64, HID)
    w2t = np.ascontiguousarray(w2.T)
    b1rs = np.ascontiguousarray(b1.reshape(HID // 128, 128).T)
    b2bc = np.ascontiguousarray(np.broadcast_to(b2, (128, OUTF)))

    if USE_FP8:
        bf_taps = list(range(4, 24))
        w18 = np.empty((128, 2, HID), dtype=np.float32)
        w18[:, 0, :] = w1tap[0:2].reshape(128, HID)
        w18[:, 1, :] = w1tap[2:4].reshape(128, HID)
        w18 = w18.astype(fp8)
        w18b = np.zeros((64, 2, HID), dtype=np.float32)
        w18b[:, 0, :] = w1tap[24]
        w18b = w18b.astype(fp8)
    else:
        bf_taps = list(range(0, 24))  # unused fallback
    w1p = np.ascontiguousarray(
        w1tap[bf_taps].reshape(KC_BF, 128, HID)).astype(bf16)
    w2t = np.ascontiguousarray(
        w2t.reshape(HID // 128, 128, OUTF)).astype(bf16)

    in_maps = []
    for cid in range(N_CORES):
        b, half = cid // 2, cid % 2
        h0 = half * ROWS_PER_CORE

        def tap_slice(k):
            dh, dw, q = KPOS[k]
            return xe[b, q::9,
                      1 + h0 + dh: 1 + h0 + dh + ROWS_PER_CORE,
                      1 + dw: 1 + dw + WIMG]

        xpair = np.empty((KC_BF, 128, ROWS_PER_CORE, WIMG), dtype=np.float32)
        for j, k in enumerate(range(4, 24, 2)):
            xpair[j, 0:64] = tap_slice(k)
            xpair[j, 64:128] = tap_slice(k + 1)
        im = {
            "xp": xpair.reshape(KC_BF, 128, ROWS_PER_CORE * WIMG).astype(bf16),
            "w1p": w1p, "w2t": w2t, "b1rs": b1rs, "b2bc": b2bc,
        }
        if USE_FP8:
            xq = np.empty((2, 128, ROWS_PER_CORE, WIMG), dtype=np.float32)
            for s in range(2):
                xq[s, 0:64] = tap_slice(2 * s)
                xq[s, 64:128] = tap_slice(2 * s + 1)
            t24 = np.asarray(tap_slice(24))
            xqb = np.stack([t24, t24])  # slot1 data x zero weights -> 0
            im["xq"] = xq.reshape(2, 128, ROWS_PER_CORE * WIMG).astype(fp8)
            im["xqb"] = xqb.reshape(2, 64, ROWS_PER_CORE * WIMG).astype(fp8)
            im["w18"] = w18
            im["w18b"] = w18b
        in_maps.append(im)
    return in_maps


def _assemble(results):
    out = np.empty((B, NTOK, OUTF), dtype=np.float32)
    for cid in range(N_CORES):
        b, half = cid // 2, cid % 2
        t0 = half * TOK_PER_CORE
        out[b, t0: t0 + TOK_PER_CORE, :] = results[cid]["out"]
    return out


def kernel(x, w1, b1, w2, b2, image_h, image_w):
    in_maps = _host_prep(x, w1, b1, w2, b2)
    nc = _build_bass()
    from concourse.bass_utils import run_bass_kernel_spmd
    res = run_bass_kernel_spmd(nc, in_maps, list(range(N_CORES)))
    return _assemble(res.results)


# revision 27
# speedup vs baseline: 1.0707x; 1.0707x over previous
"""Trainium2 Bass kernel for nn_DMlp_46823733461564 (dense_mlp).

Computes: token-grid 3x3 masked-neighborhood gather (pixel-shuffle +
reflection-pad + masked unfold, algebraically reduced to a channel-
permuted shifted gather) followed by fc1(1600->1024) + exact GELU +
fc2(1024->576).

Sharding: data-parallel over (batch, image-half) -> 8 cores, 8192 tokens
each; fc weights replicated. The gather runs on-device as strided DMAs
from a host-prepared reflection-extended channel-major image.

fc1 K-dim mixed precision: 192 of the 1600 K-rows (taps 0,1,24) run as
one fp8-e4m3 DoubleRow matmul pass (2 K-streams/cycle); the remaining
1408 K-rows (taps 2..23, 11 full 128-chunks) run in bf16. fc2 in bf16.
All w1 scaled x64 host-side (fp8 subnormal avoidance), undone by the
GELU activation's scale. Measured end-to-end rel err ~1.5e-2 (gate 2e-2).
"""
import os
import sys

import numpy as np

_TRN_REPO = "/opt/trn_rl_repo"
if _TRN_REPO not in sys.path:
    sys.path.insert(0, _TRN_REPO)

B, HIMG, WIMG = 4, 128, 128
C = 64
L = 576           # C * 9
NTOK = HIMG * WIMG
HID = 1024
OUTF = 576
INF = 1600        # C * 25
N_CORES = 8
ROWS_PER_CORE = HIMG // 2          # 64 token rows
TOK_PER_CORE = ROWS_PER_CORE * WIMG  # 8192
TILE_ROWS = 4                      # image rows per token tile
TT = TILE_ROWS * WIMG              # 512 tokens per tile
N_TILES = ROWS_PER_CORE // TILE_ROWS  # 16
KC_BF = 11                         # bf16 K-chunks (taps 2..23)
W1SCALE = 64.0

_MASK = np.array([
    [1, 0, 0, 1, 0, 0, 1],
    [0, 1, 0, 1, 0, 1, 0],
    [0, 0, 1, 1, 1, 0, 0],
    [1, 1, 1, 1, 1, 1, 1],
    [0, 0, 1, 1, 1, 0, 0],
    [0, 1, 0, 1, 0, 1, 0],
    [1, 0, 0, 1, 0, 0, 1]], dtype=bool)
MASK_POS = [(i, j) for i in range(7) for j in range(7) if _MASK[i, j]]


def _dmap(d):
    if d <= 1:
        return -1, d + 1
    if d <= 4:
        return 0, d - 2
    return 1, d - 5


KPOS = []
for (_di, _dj) in MASK_POS:
    _dh, _r1 = _dmap(_di)
    _dw, _r2 = _dmap(_dj)
    KPOS.append((_dh, _dw, _r1 * 3 + _r2))


def _swap_map(a, b, which):
    ch = np.arange(L)
    c, rem = ch // 9, ch % 9
    r1, r2 = rem // 3, rem % 3
    r = r1 if which == 0 else r2
    rs = np.where(r == a, b, np.where(r == b, a, r))
    if which == 0:
        return c * 9 + rs * 3 + r2
    return c * 9 + r1 * 3 + rs


def _build_xe(x):
    """x: (B, NTOK, L) -> xe: (B, L, HIMG+2, WIMG+2) reflection-extended,
    channel-permuted borders."""
    xt = np.ascontiguousarray(x.transpose(0, 2, 1)).reshape(B, L, HIMG, WIMG)
    xe = np.empty((B, L, HIMG + 2, WIMG + 2), dtype=np.float32)
    xe[:, :, 1:-1, 1:-1] = xt
    xe[:, :, 0, 1:-1] = xt[:, _swap_map(1, 2, 0), 0, :]
    xe[:, :, -1, 1:-1] = xt[:, _swap_map(0, 1, 0), -1, :]
    xe[:, :, :, 0] = np.take(xe[:, :, :, 1], _swap_map(1, 2, 1), axis=1)
    xe[:, :, :, -1] = np.take(xe[:, :, :, -2], _swap_map(0, 1, 1), axis=1)
    return xe


_NC_CACHE = {}

USE_FP8 = os.environ.get("KERNEL_FP8", "1") == "1"
MODE = "bf16+fp8" if USE_FP8 else "bf16"

_WS_COUNTER = [0]


def _split_waits(nc, limit=1):
    """walrus in this toolchain accepts only ONE sync wait per instruction;
    move excess waits onto same-engine NoOps inserted just before (engine
    program order makes this equivalent)."""
    import concourse.mybir as mybir

    def noop(engine, waits):
        _WS_COUNTER[0] += 1
        return mybir.InstNoOp(
            name=f"WS-{_WS_COUNTER[0]}",
            sync_info=mybir.SyncInfo(on_wait=list(waits), on_update=[]),
            bass_nofuse=True,
            engine=engine,
        )

    for fn in nc.m.functions:
        for blk in fn.blocks:
            new_insts = []
            for inst in blk.instructions:
                si = getattr(inst, "sync_info", None)
                waits = list(si.on_wait) if si and si.on_wait else []
                if len(waits) > limit:
                    excess = waits[: len(waits) - limit]
                    si.on_wait = waits[len(waits) - limit:]
                    while excess:
                        new_insts.append(noop(inst.engine, excess[:limit]))
                        excess = excess[limit:]
                new_insts.append(inst)
            blk.instructions = new_insts


def _build_bass():
    if "nc" in _NC_CACHE:
        return _NC_CACHE["nc"]
    import concourse.bass as bass
    import concourse.mybir as mybir
    from concourse.tile import TileContext

    f32 = mybir.dt.float32
    bf16 = mybir.dt.bfloat16
    fp8 = mybir.dt.float8e4
    AF = mybir.ActivationFunctionType
    Alu = mybir.AluOpType
    DR = mybir.MatmulPerfMode.DoubleRow

    nc = bass.Bass("TRN2", target_bir_lowering=False, debug=False)
    xp = nc.dram_tensor("xp", (KC_BF, 128, ROWS_PER_CORE * WIMG), bf16,
                        kind="ExternalInput")
    w1p = nc.dram_tensor("w1p", (KC_BF, 128, HID), bf16, kind="ExternalInput")
    w2t = nc.dram_tensor("w2t", (HID // 128, 128, OUTF), bf16,
                         kind="ExternalInput")
    b1rs = nc.dram_tensor("b1rs", (128, HID // 128), f32, kind="ExternalInput")
    b2bc = nc.dram_tensor("b2bc", (128, OUTF), f32, kind="ExternalInput")
    if USE_FP8:
        xq = nc.dram_tensor("xq", (2, 128, ROWS_PER_CORE * WIMG), fp8,
                            kind="ExternalInput")
        w18 = nc.dram_tensor("w18", (128, 2, HID), fp8, kind="ExternalInput")
    out = nc.dram_tensor("out", (TOK_PER_CORE, OUTF), f32,
                         kind="ExternalOutput")

    with TileContext(nc) as tc:
        with (
            tc.tile_pool(name="wpool", bufs=1) as wpool,
            tc.tile_pool(name="fpool", bufs=3) as fpool,
            tc.tile_pool(name="hpool", bufs=2) as hpool,
            tc.tile_pool(name="opool", bufs=3) as opool,
            tc.tile_pool(name="ps1", bufs=2, space="PSUM") as ps1,
            tc.tile_pool(name="ps2", bufs=2, space="PSUM") as ps2,
        ):
            # --- PE warmup: dependency-free matmuls fill the initial DMA
            # wait and push the HAM clock gate to 8/8 before real work ---
            n_warm = int(os.environ.get("KERNEL_WARMUP", "320"))
            warm_n = int(os.environ.get("KERNEL_WARMUP_N", "128"))
            if n_warm:
                warm = wpool.tile([128, 512], bf16, tag="warm")
                nc.vector.memset(warm[:, :], 0.0)
                wps = ps2.tile([128, 512], f32, tag="poa")
                for _ in range(n_warm):
                    nc.tensor.matmul(wps[:, 0:warm_n], warm[:, 0:128],
                                     warm[:, 0:warm_n], start=True, stop=True)
            # --- replicated weights, loaded once. Merged into few DMA
            # instructions (each dma_start costs ~0.6us of queue time) and
            # spread over the idle gpsimd/vector queues so the streams run
            # in parallel and never crowd the gather queues ---
            # --- tile-0 feat gathers first: per-chunk DMAs (chunk-granular
            # semaphores let fc1 m=0 start as chunks land), spread over the
            # sync+scalar queues (each dynamic queue streams ~100GB/s) ---
            QS = [nc.sync, nc.scalar, nc.gpsimd]
            t0_fts = []
            for j in range(KC_BF):
                ft = fpool.tile([128, TT], bf16, tag=f"f{j}")
                t0_fts.append(ft)
                QS[j % 2].dma_start(
                    out=ft[:, :].rearrange("p (r w) -> p r w", r=TILE_ROWS),
                    in_=xp[j, :, 0:TT].rearrange(
                        "p (r w) -> p r w", r=TILE_ROWS))
            t0_ft8 = None
            if USE_FP8:
                w18t = wpool.tile([128, 2, HID], fp8, tag="w18")
                nc.gpsimd.dma_start(out=w18t[:, :, :], in_=w18[:, :, :])
                t0_ft8 = fpool.tile([128, 2, TT], fp8, tag="f8")
                nc.scalar.dma_start(
                    out=t0_ft8[:, :, :],
                    in_=xq[:, :, 0:TT].rearrange("s p t -> p s t"))
            # w1 in half-HID chunks over all three queues, m=0..3 halves
            # first so early fc1 m-groups never wait on late weight bytes
            w1all = wpool.tile([128, KC_BF, HID], bf16, tag="w1")
            HH = HID // 2
            for half in range(2):
                hs = slice(half * HH, half * HH + HH)
                for j in range(KC_BF):
                    QS[(j + half) % 3].dma_start(
                        out=w1all[:, j, hs], in_=w1p[j, :, hs])
            w1sb = [w1all[:, j, :] for j in range(KC_BF)]
            b1t = wpool.tile([128, HID // 128], f32, tag="b1")
            nc.gpsimd.dma_start(out=b1t[:, :], in_=b1rs[:, :])
            # fc2 weights + b2 aren't needed until ~25us in
            w2all = wpool.tile([128, HID // 128, OUTF], bf16, tag="w2")
            w2sb = [w2all[:, j, :] for j in range(HID // 128)]
            b2t = None

            def emit_fc2(hts, r0, tt):
                # --- fc2: out[tok, :] = h.T @ w2t + b2 ---
                NH = OUTF // 2
                ns = tt // 128
                ot = opool.tile([128, ns, OUTF], f32, tag="o")
                for s in range(ns):
                    poa = ps2.tile([128, NH], f32, tag="poa")
                    pob = ps2.tile([128, NH], f32, tag="pob")
                    for j in range(HID // 128):
                        nc.tensor.matmul(
                            poa[:, :],
                            hts[j][:, s * 128: (s + 1) * 128],
                            w2sb[j][:, 0:NH],
                            start=(j == 0), stop=(j == HID // 128 - 1),
                        )
                        nc.tensor.matmul(
                            pob[:, :],
                            hts[j][:, s * 128: (s + 1) * 128],
                            w2sb[j][:, NH:OUTF],
                            start=(j == 0), stop=(j == HID // 128 - 1),
                        )
                    nc.vector.tensor_tensor(
                        out=ot[:, s, 0:NH], in0=poa[:, :], in1=b2t[:, 0:NH],
                        op=Alu.add)
                    nc.vector.tensor_tensor(
                        out=ot[:, s, NH:OUTF], in0=pob[:, :],
                        in1=b2t[:, NH:OUTF], op=Alu.add)
                tok0 = r0 * WIMG
                nc.gpsimd.dma_start(
                    out=out[tok0: tok0 + tt, :].rearrange(
                        "(s p) f -> p s f", s=ns),
                    in_=ot[:, :, :])

            # tile list: 4-row tiles, with the last tile split in half so
            # the serial fc2 tail after the final fc1 is shorter
            tiles = [(i * TILE_ROWS, TILE_ROWS) for i in range(N_TILES - 1)]
            tiles += [((N_TILES - 1) * TILE_ROWS, TILE_ROWS // 2),
                      ((N_TILES - 1) * TILE_ROWS + TILE_ROWS // 2,
                       TILE_ROWS // 2)]

            prev = None  # (hts, r0, tt) of the previous tile: fc2 runs one
            # tile behind fc1 so the PE never waits on the GELU latency
            for t_i, (r0, nrows) in enumerate(tiles):
                tt = nrows * WIMG
                tok = slice(r0 * WIMG, r0 * WIMG + tt)
                if t_i == 0:
                    fts, ft8 = t0_fts, t0_ft8
                    b2t = wpool.tile([128, OUTF], f32, tag="b2")
                    nc.gpsimd.dma_start(out=b2t[:, :], in_=b2bc[:, :])
                    nc.gpsimd.dma_start(
                        out=w2all[:, :, :],
                        in_=w2t[:, :, :].rearrange("j p f -> p j f"))
                else:
                    # steady state: one merged DMA for the 10 bf16 chunks,
                    # one per fp8 slot pair
                    ftall = fpool.tile([128, KC_BF, tt], bf16, tag="f")
                    nc.sync.dma_start(
                        out=ftall[:, :, :],
                        in_=xp[:, :, tok].rearrange("j p t -> p j t"))
                    fts = [ftall[:, j, :] for j in range(KC_BF)]
                    if USE_FP8:
                        ft8 = fpool.tile([128, 2, tt], fp8, tag="f8")
                        nc.scalar.dma_start(
                            out=ft8[:, :, :],
                            in_=xq[:, :, tok].rearrange("s p t -> p s t"))
                # --- fc1 + GELU: h[m] = gelu((w1.T @ featT)/64 + b1) ---
                hts = []
                for m in range(HID // 128):
                    ps = ps1.tile([128, tt], f32)
                    if USE_FP8:
                        nc.tensor.matmul(
                            ps[:, :], w18t[:, :, m * 128:(m + 1) * 128],
                            ft8[:, :, :], start=True, stop=False,
                            perf_mode=DR)
                        for j in range(KC_BF):
                            nc.tensor.matmul(
                                ps[:, :], w1sb[j][:, m * 128:(m + 1) * 128],
                                fts[j][:, :],
                                start=False, stop=(j == KC_BF - 1))
                    else:
                        for j in range(KC_BF):
                            nc.tensor.matmul(
                                ps[:, :], w1sb[j][:, m * 128:(m + 1) * 128],
                                fts[j][:, :],
                                start=(j == 0), stop=(j == KC_BF - 1))
                    ht = hpool.tile([128, tt], bf16, tag=f"h{m}")
                    nc.scalar.activation(ht[:, :], ps[:, :], AF.Gelu,
                                         bias=b1t[:, m: m + 1],
                                         scale=1.0 / W1SCALE)
                    hts.append(ht)
                if prev is not None:
                    emit_fc2(*prev)
                prev = (hts, r0, tt)
            emit_fc2(*prev)

    _split_waits(nc)
    _NC_CACHE["nc"] = nc
    return nc


def _host_prep(x, w1, b1, w2, b2):
    import ml_dtypes
    bf16 = ml_dtypes.bfloat16
    fp8 = ml_dtypes.float8_e4m3

    x = np.ascontiguousarray(np.asarray(x, dtype=np.float32))
    w1 = np.asarray(w1, dtype=np.float32)
    b1 = np.asarray(b1, dtype=np.float32)
    w2 = np.asarray(w2, dtype=np.float32)
    b2 = np.asarray(b2, dtype=np.float32)

    xe = _build_xe(x)
    # w1 rows in tap-major order: row k*64+c <- w1.T row c*25+k; scaled x64
    w1t = np.ascontiguousarray(w1.T) * W1SCALE  # (1600, 1024)
    w1tap = np.ascontiguousarray(
        w1t.reshape(C, 25, HID).transpose(1, 0, 2))  # (25, 64, HID)
    w2t = np.ascontiguousarray(w2.T)
    b1rs = np.ascontiguousarray(b1.reshape(HID // 128, 128).T)
    b2bc = np.ascontiguousarray(np.broadcast_to(b2, (128, OUTF)))

    if USE_FP8:
        bf_taps = list(range(2, 24))
        w18 = np.zeros((128, 2, HID), dtype=np.float32)
        w18[:, 0, :] = w1tap[0:2].reshape(128, HID)
        w18[0:64, 1, :] = w1tap[24]
        w18 = w18.astype(fp8)
    else:
        bf_taps = list(range(0, 24))  # unused fallback
    w1p = np.ascontiguousarray(
        w1tap[bf_taps].reshape(KC_BF, 128, HID)).astype(bf16)
    w2t = np.ascontiguousarray(
        w2t.reshape(HID // 128, 128, OUTF)).astype(bf16)

    in_maps = []
    for cid in range(N_CORES):
        b, half = cid // 2, cid % 2
        h0 = half * ROWS_PER_CORE

        def tap_slice(k):
            dh, dw, q = KPOS[k]
            return xe[b, q::9,
                      1 + h0 + dh: 1 + h0 + dh + ROWS_PER_CORE,
                      1 + dw: 1 + dw + WIMG]

        xpair = np.empty((KC_BF, 128, ROWS_PER_CORE, WIMG), dtype=np.float32)
        for j, k in enumerate(range(2, 24, 2)):
            xpair[j, 0:64] = tap_slice(k)
            xpair[j, 64:128] = tap_slice(k + 1)
        im = {
            "xp": xpair.reshape(KC_BF, 128, ROWS_PER_CORE * WIMG).astype(bf16),
            "w1p": w1p, "w2t": w2t, "b1rs": b1rs, "b2bc": b2bc,
        }
        if USE_FP8:
            xq = np.zeros((2, 128, ROWS_PER_CORE, WIMG), dtype=np.float32)
            xq[0, 0:64] = tap_slice(0)
            xq[0, 64:128] = tap_slice(1)
            xq[1, 0:64] = tap_slice(24)
            im["xq"] = xq.reshape(2, 128, ROWS_PER_CORE * WIMG).astype(fp8)
            im["w18"] = w18
        in_maps.append(im)
    return in_maps


def _assemble(results):
    out = np.empty((B, NTOK, OUTF), dtype=np.float32)
    for cid in range(N_CORES):
        b, half = cid // 2, cid % 2
        t0 = half * TOK_PER_CORE
        out[b, t0: t0 + TOK_PER_CORE, :] = results[cid]["out"]
    return out


def kernel(x, w1, b1, w2, b2, image_h, image_w):
    in_maps = _host_prep(x, w1, b1, w2, b2)
    nc = _build_bass()
    from concourse.bass_utils import run_bass_kernel_spmd
    res = run_bass_kernel_spmd(nc, in_maps, list(range(N_CORES)))
    return _assemble(res.results)
